# revision 23
# baseline (speedup 1.0000x reference)
"""CELPNetSub kernel for Trainium2 (8 NeuronCores, pure data-parallel).

Reference computation (per batch row):
    tmp = tanh(concat(cond, prev) @ w_d1.T + b_d1)
    tmp = tanh(tmp @ w_d2.T + b_d2)
    g1 = GRUCell(tmp, h1); g2 = GRUCell(g1, h2); g3 = GRUCell(g2, h3)
    out = tanh(g3 @ w_out.T + b_out)
    return out, g1, g2, g3

Shapes: B=16384, S=40, C=256.  Sharding: batch across 8 cores (2048/core),
weights replicated.  On-chip layout is feature-major ([C, B_tile] tiles,
B_tile=512) so the PE contracts over the partition dim; batch-major DRAM
inputs are transposed on the PE, outputs transposed back before store.
Matmuls run in float32r (~1 cycle/row at N=512, ~1.6e-4 rel err).
"""

import numpy as np

B, S, C = 16384, 40, 256
NCORES = 8
BLOC = B // NCORES          # 2048 rows per core
NT = 512                    # batch tile (free dim of matmuls)
NBT = BLOC // NT            # 4 batch tiles per core
NB = NT // 128              # 4 batch sub-blocks per tile

_CACHE = {}


def _build(repeat=1, trace_sim=False, no_in_tp=False, no_out_tp=False, no_mm=False, no_blend=False, mm2x=False):
    import concourse.bacc as bacc
    import concourse.mybir as mybir
    from concourse.tile import TileContext
    from concourse.masks import make_identity

    f32 = mybir.dt.float32
    f32r = mybir.dt.float32r
    AF = mybir.ActivationFunctionType
    ALU = mybir.AluOpType

    nc = bacc.Bacc(trn_type="TRN2", target_bir_lowering=False, debug=False)

    # --- DRAM I/O (per core shapes); activations/weights declared f32r so
    # DMA-direct loads satisfy the fp32r-rounded-producer rule ---
    cond = nc.dram_tensor("cond", [BLOC, C], f32r, kind="ExternalInput").ap()
    prev = nc.dram_tensor("prev", [BLOC, S], f32r, kind="ExternalInput").ap()
    h_in = [
        nc.dram_tensor(f"h{k+1}", [BLOC, C], f32r, kind="ExternalInput").ap()
        for k in range(3)
    ]
    wt_d1 = nc.dram_tensor("wt_d1", [C + S, C], f32r, kind="ExternalInput").ap()
    wt_d2 = nc.dram_tensor("wt_d2", [C, C], f32r, kind="ExternalInput").ap()
    wt_ih = [
        nc.dram_tensor(f"wt_ih{k+1}", [C, 3 * C], f32r, kind="ExternalInput").ap()
        for k in range(3)
    ]
    wt_hh = [
        nc.dram_tensor(f"wt_hh{k+1}", [C, 3 * C], f32r, kind="ExternalInput").ap()
        for k in range(3)
    ]
    wt_out = nc.dram_tensor("wt_out", [C, S], f32r, kind="ExternalInput").ap()
    biases = nc.dram_tensor("biases", [128, 29], f32, kind="ExternalInput").ap()

    out_d = nc.dram_tensor("out", [BLOC, S], f32, kind="ExternalOutput").ap()
    g_d = [
        nc.dram_tensor(f"g{k+1}", [BLOC, C], f32, kind="ExternalOutput").ap()
        for k in range(3)
    ]

    with TileContext(nc, trace_sim=trace_sim) as tc:
        import contextlib

        ctx = contextlib.ExitStack()
        with ctx:
            singles = ctx.enter_context(tc.tile_pool(name="singles", bufs=1))
            instage = ctx.enter_context(tc.tile_pool(name="instage", bufs=3))
            xt = ctx.enter_context(tc.tile_pool(name="xt", bufs=2))
            act = ctx.enter_context(tc.tile_pool(name="act", bufs=2))
            gpool = ctx.enter_context(tc.tile_pool(name="gpool", bufs=2))
            ost = ctx.enter_context(tc.tile_pool(name="ost", bufs=2))
            # PSUM pools
            tp_ps = ctx.enter_context(tc.tile_pool(name="tp_ps", bufs=1, space="PSUM"))
            mm_ps = ctx.enter_context(tc.tile_pool(name="mm_ps", bufs=6, space="PSUM"))
            st_ps = ctx.enter_context(tc.tile_pool(name="st_ps", bufs=1, space="PSUM"))

            # --- constants: identity, biases ---
            ident = singles.tile([128, 128], f32, tag="ident")
            make_identity(nc, ident)
            ident_r = singles.tile([128, 128], f32r, tag="identr")
            nc.vector.tensor_copy(ident_r, ident)
            bias_sb = singles.tile([128, 29], f32, tag="bias")
            nc.sync.dma_start(out=bias_sb, in_=biases)

            def bcol(j):
                return bias_sb[:, j : j + 1]

            # --- weights: DMA-direct into f32r tiles ---
            def load_w(dram_ap, rows, cols, tag):
                nchunk = (rows + 127) // 128
                wr = singles.tile([128, nchunk, cols], f32r, tag=tag)
                for kc in range(nchunk):
                    r0 = kc * 128
                    rn = min(128, rows - r0)
                    nc.sync.dma_start(
                        out=wr[:rn, kc, :], in_=dram_ap[r0 : r0 + rn, :]
                    )
                return wr

            wd1_r = load_w(wt_d1, C, C, "wd1")        # cond part, chunks 0,1
            wd1p_r = load_w(wt_d1[C : C + S, :], S, C, "wd1p")  # prev part [40,C]
            wd2_r = load_w(wt_d2, C, C, "wd2")
            wih_r = [load_w(wt_ih[k], C, 3 * C, f"wih{k}") for k in range(3)]
            whh_r = [load_w(wt_hh[k], C, 3 * C, f"whh{k}") for k in range(3)]
            wout_r = load_w(wt_out, C, S, "wout")

            # dummy PE op: absorbs the gpsimd identity dependency once
            dps = st_ps.tile([128, 128], f32, tag="stps")
            nc.tensor.transpose(dps, ident, ident)

            def MM(*a, **k):
                if no_mm:
                    return
                if mm2x:
                    k1 = dict(k)
                    k1["stop"] = False
                    nc.tensor.matmul(*a, **k1)
                    k2 = dict(k)
                    k2["start"] = False
                    nc.tensor.matmul(*a, **k2)
                    return
                nc.tensor.matmul(*a, **k)

            def body(iv):
                for t in range(NBT):
                    row0 = t * NT

                    # ---- stage A: load + transpose inputs (feature-major) ----
                    def load_xt(dram_ap, ncols, tag):
                        """dram [NT rows, ncols] batch-major -> f32r [ncols, NT]"""
                        nchunk = (ncols + 127) // 128
                        stg = instage.tile([128, NB, ncols], f32r, tag="instg")
                        nc.sync.dma_start(
                            out=stg,
                            in_=dram_ap[row0 : row0 + NT, :].rearrange(
                                "(b p) c -> p b c", b=NB
                            ),
                        )
                        tiles = []
                        for cchunk in range(nchunk):
                            c0 = cchunk * 128
                            cn = min(128, ncols - c0)
                            xr = xt.tile([128, NT], f32r, tag=f"{tag}{cchunk}")
                            if no_in_tp:
                                # timing variant: same copy volume, no PE work
                                for b in range(NB):
                                    src = stg[:cn, b, c0 : c0 + cn]
                                    dst = xr[:cn, 128 * b : 128 * b + cn]
                                    if cchunk % 2 == 0:
                                        nc.scalar.activation(dst, src, AF.Copy)
                                    else:
                                        nc.vector.tensor_copy(dst, src)
                                tiles.append(xr)
                                continue
                            ps = tp_ps.tile([128, NT], f32r, tag="tpps")
                            for b in range(NB):
                                nc.tensor.transpose(
                                    ps[:cn, 128 * b : 128 * (b + 1)],
                                    stg[:, b, c0 : c0 + cn],
                                    ident_r,
                                )
                            if cchunk % 2 == 0:
                                nc.scalar.activation(xr[:cn, :], ps[:cn, :], AF.Copy)
                            else:
                                nc.vector.tensor_copy(xr[:cn, :], ps[:cn, :])
                            tiles.append(xr)
                        return tiles

                    ct = load_xt(cond, C, "ct")
                    pt = load_xt(prev, S, "pt")
                    ht = [load_xt(h_in[k], C, f"ht{k}") for k in range(3)]

                    # ---- stage B/C: dense tanh layers ----
                    def dense_tanh(win, wprev, xin, xprev, bias_j, tag):
                        outs = []
                        for m in range(2):
                            ps = mm_ps.tile([128, NT], f32, tag="mmps")
                            mcols = slice(128 * m, 128 * (m + 1))
                            MM(
                                ps, win[:, 0, mcols], xin[0], start=True, stop=False
                            )
                            MM(
                                ps,
                                win[:, 1, mcols],
                                xin[1],
                                start=False,
                                stop=(wprev is None),
                            )
                            if wprev is not None:
                                MM(
                                    ps,
                                    wprev[:S, 0, mcols],
                                    xprev[0][:S, :],
                                    start=False,
                                    stop=True,
                                )
                            o = act.tile([128, NT], f32r, tag=f"{tag}{m}")
                            nc.scalar.activation(
                                o, ps, AF.Tanh, bias=bcol(bias_j + m)
                            )
                            outs.append(o)
                        return outs

                    tmp1 = dense_tanh(wd1_r, wd1p_r, ct, pt, 0, "tmp1")
                    tmp2 = dense_tanh(wd2_r, None, tmp1, None, 2, "tmp2")

                    # ---- stage D: GRU chain ----
                    xcur = tmp2
                    gts = []
                    for k in range(3):
                        bb = 4 + 8 * k
                        wih, whh, hk = wih_r[k], whh_r[k], ht[k]

                        def gates(colbase, bias_j, func, tag):
                            outs = []
                            for m in range(2):
                                ps = mm_ps.tile([128, NT], f32, tag="mmps")
                                mc = slice(colbase + 128 * m, colbase + 128 * (m + 1))
                                MM(ps, wih[:, 0, mc], xcur[0],
                                                 start=True, stop=False)
                                MM(ps, wih[:, 1, mc], xcur[1],
                                                 start=False, stop=False)
                                MM(ps, whh[:, 0, mc], hk[0],
                                                 start=False, stop=False)
                                MM(ps, whh[:, 1, mc], hk[1],
                                                 start=False, stop=True)
                                o = act.tile([128, NT], f32, tag=f"{tag}{m}")
                                nc.scalar.activation(o, ps, func,
                                                     bias=bcol(bias_j + m))
                                outs.append(o)
                            return outs

                        r = gates(0, bb, AF.Sigmoid, "r")
                        z = gates(C, bb + 2, AF.Sigmoid, "z")

                        gk = []
                        for m in range(2):
                            psi = mm_ps.tile([128, NT], f32, tag="mmps")
                            psh = mm_ps.tile([128, NT], f32, tag="mmps")
                            mc = slice(2 * C + 128 * m, 2 * C + 128 * (m + 1))
                            MM(psi, wih[:, 0, mc], xcur[0],
                                             start=True, stop=False)
                            MM(psi, wih[:, 1, mc], xcur[1],
                                             start=False, stop=True)
                            MM(psh, whh[:, 0, mc], hk[0],
                                             start=True, stop=False)
                            MM(psh, whh[:, 1, mc], hk[1],
                                             start=False, stop=True)
                            # tn = (psh + b_hhn) * r
                            tn = act.tile([128, NT], f32, tag=f"tn{m}")
                            nc.vector.scalar_tensor_tensor(
                                tn, psh, bcol(bb + 6 + m), r[m],
                                op0=ALU.add, op1=ALU.mult,
                            )
                            # tn <- (psi + b_ihn) + tn  (in place)
                            nc.vector.scalar_tensor_tensor(
                                tn, psi, bcol(bb + 4 + m), tn,
                                op0=ALU.add, op1=ALU.add,
                            )
                            n = act.tile([128, NT], f32, tag=f"n{m}")
                            nc.scalar.activation(n, tn, AF.Tanh)
                            # g = n + z*(h - n); sub/mul on gpsimd (SBUF-only),
                            # reusing tn as the temp
                            g = gpool.tile([128, NT], f32r, tag=f"g{k}{m}")
                            if no_blend:
                                nc.vector.tensor_copy(g, n)
                            else:
                                nc.vector.tensor_sub(tn, hk[m].bitcast(f32), n)
                                nc.vector.tensor_mul(tn, z[m], tn)
                                nc.vector.tensor_add(g, n, tn)
                            gk.append(g)
                        gts.append(gk)
                        xcur = gk

                    # ---- stage E: store g1..g3 batch-major ----
                    for k in range(3):
                        stg = ost.tile([128, NB, C], f32, tag=f"gst{k}")
                        for b in range(NB):
                            if no_out_tp:
                                src = gts[k][b % 2][:, : C].bitcast(f32)
                                if b % 2 == 0:
                                    nc.scalar.activation(stg[:, b, :], src, AF.Copy)
                                else:
                                    nc.vector.tensor_copy(stg[:, b, :], src)
                                continue
                            ps = st_ps.tile([128, C], f32r, tag="stps")
                            for cchunk in range(2):
                                nc.tensor.transpose(
                                    ps[:, 128 * cchunk : 128 * (cchunk + 1)],
                                    gts[k][cchunk][:, 128 * b : 128 * (b + 1)],
                                    ident_r,
                                )
                            if b % 2 == 0:
                                nc.scalar.activation(
                                    stg[:, b, :], ps.bitcast(f32), AF.Copy
                                )
                            else:
                                nc.vector.tensor_copy(stg[:, b, :], ps.bitcast(f32))
                        nc.sync.dma_start(
                            out=g_d[k][row0 : row0 + NT, :].rearrange(
                                "(b p) c -> p b c", b=NB
                            ),
                            in_=stg,
                        )

                    # ---- stage F: output layer ----
                    pso = mm_ps.tile([128, NT], f32, tag="mmps")
                    MM(pso[:S, :], wout_r[:, 0, :], gts[2][0],
                                     start=True, stop=False)
                    MM(pso[:S, :], wout_r[:, 1, :], gts[2][1],
                                     start=False, stop=True)
                    ot = act.tile([128, NT], f32, tag="ot")
                    nc.scalar.activation(ot[:S, :], pso[:S, :], AF.Tanh,
                                         bias=bias_sb[:S, 28:29])
                    stg = ost.tile([128, NB, S], f32, tag="ostg")
                    for b in range(NB):
                        ps = st_ps.tile([128, C], f32, tag="stps")
                        nc.tensor.transpose(
                            ps[:, :S], ot[:S, 128 * b : 128 * (b + 1)],
                            ident[:S, :S],
                        )
                        nc.scalar.activation(stg[:, b, :], ps[:, :S], AF.Copy)
                    nc.sync.dma_start(
                        out=out_d[row0 : row0 + NT, :].rearrange(
                            "(b p) c -> p b c", b=NB
                        ),
                        in_=stg,
                    )

            if repeat == 1:
                body(None)
            else:
                with tc.For_i(0, repeat, 1) as iv:
                    body(iv)

    nc.finalize()
    return nc


def _prep_host(inputs):
    """Shard activations, pre-transpose weights, pack biases."""
    f = lambda x: np.ascontiguousarray(np.asarray(x, dtype=np.float32))
    wT = {
        "wt_d1": f(inputs["w_d1"].T),
        "wt_d2": f(inputs["w_d2"].T),
        "wt_out": f(inputs["w_out"].T),
    }
    for k in (1, 2, 3):
        wT[f"wt_ih{k}"] = f(inputs[f"w_ih{k}"].T)
        wT[f"wt_hh{k}"] = f(inputs[f"w_hh{k}"].T)

    bias = np.zeros((128, 29), np.float32)
    bias[:, 0] = inputs["b_d1"][0:128]
    bias[:, 1] = inputs["b_d1"][128:256]
    bias[:, 2] = inputs["b_d2"][0:128]
    bias[:, 3] = inputs["b_d2"][128:256]
    for k in range(3):
        bih = np.asarray(inputs[f"b_ih{k+1}"], np.float32)
        bhh = np.asarray(inputs[f"b_hh{k+1}"], np.float32)
        brz = bih[: 2 * C] + bhh[: 2 * C]
        bb = 4 + 8 * k
        for m in range(4):
            bias[:, bb + m] = brz[128 * m : 128 * (m + 1)]
        bias[:, bb + 4] = bih[2 * C : 2 * C + 128]
        bias[:, bb + 5] = bih[2 * C + 128 :]
        bias[:, bb + 6] = bhh[2 * C : 2 * C + 128]
        bias[:, bb + 7] = bhh[2 * C + 128 :]
    bias[:S, 28] = inputs["b_out"]

    in_maps = []
    for c in range(NCORES):
        sl = slice(c * BLOC, (c + 1) * BLOC)
        m = {
            "cond": f(inputs["cond"][sl]),
            "prev": f(inputs["prev"][sl]),
            "h1": f(inputs["h1"][sl]),
            "h2": f(inputs["h2"][sl]),
            "h3": f(inputs["h3"][sl]),
            "biases": bias,
        }
        m.update(wT)
        in_maps.append(m)
    return in_maps


def kernel(**inputs):
    from concourse.bass_utils import run_bass_kernel_spmd

    if "nc" not in _CACHE:
        _CACHE["nc"] = _build(repeat=1)
    nc = _CACHE["nc"]
    in_maps = _prep_host(inputs)
    res = run_bass_kernel_spmd(nc, in_maps, list(range(NCORES)))
    outs = res.results
    out = np.concatenate([outs[c]["out"] for c in range(NCORES)], axis=0)
    g1 = np.concatenate([outs[c]["g1"] for c in range(NCORES)], axis=0)
    g2 = np.concatenate([outs[c]["g2"] for c in range(NCORES)], axis=0)
    g3 = np.concatenate([outs[c]["g3"] for c in range(NCORES)], axis=0)
    return out, g1, g2, g3


# revision 41
# speedup vs baseline: 1.7912x; 1.7912x over previous
"""CELPNetSub kernel for Trainium2 (8 NeuronCores, pure data-parallel).

Reference computation (per batch row):
    tmp = tanh(concat(cond, prev) @ w_d1.T + b_d1)
    tmp = tanh(tmp @ w_d2.T + b_d2)
    g1 = GRUCell(tmp, h1); g2 = GRUCell(g1, h2); g3 = GRUCell(g2, h3)
    out = tanh(g3 @ w_out.T + b_out)
    return out, g1, g2, g3

Shapes: B=16384, S=40, C=256.  Sharding: batch across 8 cores (2048/core),
weights replicated.  On-chip layout is feature-major ([C, B_tile] tiles,
B_tile=512) so the PE contracts over the partition dim; batch-major DRAM
inputs are transposed on the PE, outputs transposed back before store.
Matmuls run in float32r (~1 cycle/row at N=512, ~1.6e-4 rel err).
"""

import numpy as np

B, S, C = 16384, 40, 256
NCORES = 8
BLOC = B // NCORES          # 2048 rows per core
NT = 512                    # batch tile (free dim of matmuls)
NBT = BLOC // NT            # 4 batch tiles per core
NB = NT // 128              # 4 batch sub-blocks per tile

_CACHE = {}


def _build(repeat=1, trace_sim=False, no_in_tp=False, no_out_tp=False, no_mm=False, no_blend=False, mm2x=False, no_in_dma=False, no_out_dma=False):
    import concourse.bacc as bacc
    import concourse.mybir as mybir
    from concourse.tile import TileContext

    f32 = mybir.dt.float32
    f32r = mybir.dt.float32r
    AF = mybir.ActivationFunctionType
    ALU = mybir.AluOpType

    nc = bacc.Bacc(trn_type="TRN2", target_bir_lowering=False, debug=False)

    # --- DRAM I/O (per core shapes); activations/weights declared f32r so
    # DMA-direct loads satisfy the fp32r-rounded-producer rule ---
    cond = nc.dram_tensor("cond", [BLOC, C], f32r, kind="ExternalInput").ap()
    prev = nc.dram_tensor("prev", [BLOC, S], f32r, kind="ExternalInput").ap()
    h_in = [
        nc.dram_tensor(f"h{k+1}", [BLOC, C], f32r, kind="ExternalInput").ap()
        for k in range(3)
    ]
    wt_d1 = nc.dram_tensor("wt_d1", [C + S, C], f32r, kind="ExternalInput").ap()
    wt_d2 = nc.dram_tensor("wt_d2", [C, C], f32r, kind="ExternalInput").ap()
    wt_ih = [
        nc.dram_tensor(f"wt_ih{k+1}", [C, 3 * C], f32r, kind="ExternalInput").ap()
        for k in range(3)
    ]
    wt_hh = [
        nc.dram_tensor(f"wt_hh{k+1}", [C, 3 * C], f32r, kind="ExternalInput").ap()
        for k in range(3)
    ]
    wt_out = nc.dram_tensor("wt_out", [C, S], f32r, kind="ExternalInput").ap()
    biases = nc.dram_tensor("biases", [128, 35], f32, kind="ExternalInput").ap()
    ident_d = nc.dram_tensor("ident", [128, 128], f32r, kind="ExternalInput").ap()

    out_d = nc.dram_tensor("out", [BLOC, S], f32, kind="ExternalOutput").ap()
    g_d = [
        nc.dram_tensor(f"g{k+1}", [BLOC, C], f32, kind="ExternalOutput").ap()
        for k in range(3)
    ]

    with TileContext(nc, trace_sim=trace_sim) as tc:
        import contextlib

        ctx = contextlib.ExitStack()
        with ctx:
            singles = ctx.enter_context(tc.tile_pool(name="singles", bufs=1))
            instage = ctx.enter_context(tc.tile_pool(name="instage", bufs=2))
            insth = ctx.enter_context(tc.tile_pool(name="insth", bufs=4))
            xt = ctx.enter_context(tc.tile_pool(name="xt", bufs=2))
            act = ctx.enter_context(tc.tile_pool(name="act", bufs=2))
            gpool = ctx.enter_context(tc.tile_pool(name="gpool", bufs=2))
            ost = ctx.enter_context(tc.tile_pool(name="ost", bufs=1))
            # PSUM pools
            tp_ps = ctx.enter_context(tc.tile_pool(name="tp_ps", bufs=1, space="PSUM"))
            mm_ps = ctx.enter_context(tc.tile_pool(name="mm_ps", bufs=4, space="PSUM"))
            dd_ps = ctx.enter_context(tc.tile_pool(name="dd_ps", bufs=2, space="PSUM"))
            st_ps = ctx.enter_context(tc.tile_pool(name="st_ps", bufs=1, space="PSUM"))

            # --- constants: identity (host-provided), biases ---
            ident_r = singles.tile([128, 128], f32r, tag="identr")
            nc.sync.dma_start(out=ident_r, in_=ident_d)
            ident = ident_r.bitcast(f32)
            bias_sb = singles.tile([128, 35], f32, tag="bias")
            nc.sync.dma_start(out=bias_sb, in_=biases)

            def bcol(j):
                return bias_sb[:, j : j + 1]

            # --- weights: DMA-direct into f32r tiles ---
            def load_w(dram_ap, rows, cols, tag):
                nchunk = (rows + 127) // 128
                wr = singles.tile([128, nchunk, cols], f32r, tag=tag)
                for kc in range(nchunk):
                    r0 = kc * 128
                    rn = min(128, rows - r0)
                    nc.sync.dma_start(
                        out=wr[:rn, kc, :], in_=dram_ap[r0 : r0 + rn, :]
                    )
                return wr

            wd1_r = load_w(wt_d1, C, C, "wd1")        # cond part, chunks 0,1
            wd1p_r = load_w(wt_d1[C : C + S, :], S, C, "wd1p")  # prev part [40,C]
            wd2_r = load_w(wt_d2, C, C, "wd2")
            wih_r = [load_w(wt_ih[k], C, 3 * C, f"wih{k}") for k in range(3)]
            whh_r = [load_w(wt_hh[k], C, 3 * C, f"whh{k}") for k in range(3)]
            wout_r = load_w(wt_out, C, S, "wout")

            def MM(*a, **k):
                if no_mm:
                    return
                if mm2x:
                    k1 = dict(k)
                    k1["stop"] = False
                    nc.tensor.matmul(*a, **k1)
                    k2 = dict(k)
                    k2["start"] = False
                    nc.tensor.matmul(*a, **k2)
                    return
                nc.tensor.matmul(*a, **k)

            def body(iv):
                pending = [None]
                for t in range(NBT):
                    row0 = t * NT

                    # ---- stage A: load + transpose inputs (feature-major) ----
                    def load_xt(dram_ap, ncols, tag):
                        """dram [NT rows, ncols] batch-major -> f32r [ncols, NT]"""
                        nchunk = (ncols + 127) // 128
                        pool = insth if tag.startswith("ht") else instage
                        stg = pool.tile([128, NB, ncols], f32r,
                                        tag="inh" if tag.startswith("ht") else f"in{tag}")
                        if no_in_dma:
                            nc.sync.dma_start(out=stg[:, :1, :1],
                                              in_=dram_ap[row0 : row0 + 128, :1])
                        else:
                            nc.sync.dma_start(
                                out=stg,
                                in_=dram_ap[row0 : row0 + NT, :].rearrange(
                                    "(b p) c -> p b c", b=NB
                                ),
                            )
                        tiles = []
                        for cchunk in range(nchunk):
                            c0 = cchunk * 128
                            cn = min(128, ncols - c0)
                            xr = xt.tile([128, NT], f32r, tag=f"{tag}{cchunk}")
                            if no_in_tp:
                                # timing variant: same copy volume, no PE work
                                for b in range(NB):
                                    src = stg[:cn, b, c0 : c0 + cn]
                                    dst = xr[:cn, 128 * b : 128 * b + cn]
                                    if cchunk % 2 == 0:
                                        nc.scalar.activation(dst, src, AF.Copy)
                                    else:
                                        nc.vector.tensor_copy(dst, src)
                                tiles.append(xr)
                                continue
                            ps = tp_ps.tile([128, NT], f32r, tag="tpps")
                            for b in range(NB):
                                nc.tensor.transpose(
                                    ps[:cn, 128 * b : 128 * (b + 1)],
                                    stg[:, b, c0 : c0 + cn],
                                    ident_r,
                                )
                            if cchunk % 2 == 0:
                                nc.scalar.activation(xr[:cn, :], ps[:cn, :], AF.Copy)
                            else:
                                nc.vector.tensor_copy(xr[:cn, :], ps[:cn, :])
                            tiles.append(xr)
                        return tiles

                    ct = load_xt(cond, C, "ct")
                    pt = load_xt(prev, S, "pt")
                    ht = [load_xt(h_in[k], C, f"ht{k}") for k in range(3)]

                    # ---- stage B/C: dense tanh layers ----
                    def dense_tanh(win, wprev, xin, xprev, bias_j, tag):
                        outs = []
                        for m in range(2):
                            ps = dd_ps.tile([128, NT], f32, tag="ddps")
                            mcols = slice(128 * m, 128 * (m + 1))
                            MM(
                                ps, win[:, 0, mcols], xin[0], start=True, stop=False
                            )
                            MM(
                                ps,
                                win[:, 1, mcols],
                                xin[1],
                                start=False,
                                stop=(wprev is None),
                            )
                            if wprev is not None:
                                MM(
                                    ps,
                                    wprev[:S, 0, mcols],
                                    xprev[0][:S, :],
                                    start=False,
                                    stop=True,
                                )
                            o = act.tile([128, NT], f32r, tag=f"{tag}{m}")
                            nc.scalar.activation(
                                o, ps, AF.Tanh, bias=bcol(bias_j + m)
                            )
                            outs.append(o)
                        return outs

                    tmp1 = dense_tanh(wd1_r, wd1p_r, ct, pt, 0, "tmp1")
                    tmp2 = dense_tanh(wd2_r, None, tmp1, None, 2, "tmp2")

                    # ---- stage D: GRU chain ----
                    xcur = tmp2
                    gts = []
                    for k in range(3):
                        bb = 4 + 8 * k
                        wih, whh, hk = wih_r[k], whh_r[k], ht[k]

                        def gates(colbase, bias_j, func, tag):
                            outs = []
                            for m in range(2):
                                ps = mm_ps.tile([128, NT], f32, tag="mmps")
                                mc = slice(colbase + 128 * m, colbase + 128 * (m + 1))
                                MM(ps, whh[:, 0, mc], hk[0],
                                                 start=True, stop=False)
                                MM(ps, whh[:, 1, mc], hk[1],
                                                 start=False, stop=False)
                                MM(ps, wih[:, 0, mc], xcur[0],
                                                 start=False, stop=False)
                                MM(ps, wih[:, 1, mc], xcur[1],
                                                 start=False, stop=True)
                                o = act.tile([128, NT], f32, tag=f"{tag}{m}")
                                nc.scalar.activation(o, ps, func,
                                                     bias=bcol(bias_j + m))
                                outs.append(o)
                            return outs

                        # per-m: r matmuls+sigmoid then this m's n-gate
                        # psums, so the m0 chain starts as early as possible;
                        # z's PE work last (needed only at the blend)
                        r, psns = [], []
                        for m in range(2):
                            # h-only matmuls first: r's hh part and the full
                            # n-gate hh psum are ready before xcur arrives
                            ps = mm_ps.tile([128, NT], f32, tag="mmps")
                            mc = slice(128 * m, 128 * (m + 1))
                            MM(ps, whh[:, 0, mc], hk[0],
                                             start=True, stop=False)
                            MM(ps, whh[:, 1, mc], hk[1],
                                             start=False, stop=False)
                            psh = mm_ps.tile([128, NT], f32, tag="mmps")
                            mcn = slice(2 * C + 128 * m, 2 * C + 128 * (m + 1))
                            MM(psh, whh[:, 0, mcn], hk[0],
                                             start=True, stop=False)
                            MM(psh, whh[:, 1, mcn], hk[1],
                                             start=False, stop=True)
                            MM(ps, wih[:, 0, mc], xcur[0],
                                             start=False, stop=False)
                            MM(ps, wih[:, 1, mc], xcur[1],
                                             start=False, stop=True)
                            ro = act.tile([128, NT], f32, tag=f"r{m}")
                            nc.scalar.activation(ro, ps, AF.Sigmoid,
                                                 bias=bcol(bb + m))
                            r.append(ro)
                            psi = mm_ps.tile([128, NT], f32, tag="mmps")
                            MM(psi, wih[:, 0, mcn], xcur[0],
                                             start=True, stop=False)
                            MM(psi, wih[:, 1, mcn], xcur[1],
                                             start=False, stop=True)
                            psns.append((psi, psh))
                        z = gates(C, bb + 2, AF.Sigmoid, "z")

                        gk = []
                        for m in range(2):
                            psi, psh = psns[m]
                            # tn = (psh + b_hhn) * r
                            tn = act.tile([128, NT], f32, tag=f"tn{m}")
                            nc.vector.scalar_tensor_tensor(
                                tn, psh, bcol(bb + 6 + m), r[m],
                                op0=ALU.add, op1=ALU.mult,
                            )
                            # tn <- (psi + b_ihn) + tn  (in place)
                            nc.vector.scalar_tensor_tensor(
                                tn, psi, bcol(bb + 4 + m), tn,
                                op0=ALU.add, op1=ALU.add,
                            )
                            n = act.tile([128, NT], f32, tag=f"n{m}")
                            nc.scalar.activation(n, tn, AF.Tanh)
                            # g = n + z*(h - n); sub/mul on gpsimd (SBUF-only),
                            # reusing tn as the temp
                            g = gpool.tile([128, NT], f32r, tag=f"g{k}{m}")
                            if no_blend:
                                nc.vector.tensor_copy(g, n)
                            else:
                                nc.vector.tensor_sub(tn, hk[m].bitcast(f32), n)
                                nc.vector.tensor_mul(tn, z[m], tn)
                                nc.vector.tensor_add(g, n, tn)
                            gk.append(g)
                        gts.append(gk)
                        xcur = gk

                    # ---- stage E/F: stores deferred one B-tile so they
                    # don't steal scheduler priority from the next tile's
                    # critical path ----
                    def emit_stores(gts, row0):
                      for k in range(3):
                        stg = ost.tile([128, NB, C], f32, tag=f"gst{k}")
                        for b in range(NB):
                            if no_out_tp:
                                srcx = gts[k][b % 2][:, : C].bitcast(f32)
                                if b % 2 == 0:
                                    nc.scalar.activation(stg[:, b, :], srcx, AF.Copy)
                                else:
                                    nc.vector.tensor_copy(stg[:, b, :], srcx)
                                continue
                            ps = st_ps.tile([128, C], f32r, tag="stps")
                            for cchunk in range(2):
                                nc.tensor.transpose(
                                    ps[:, 128 * cchunk : 128 * (cchunk + 1)],
                                    gts[k][cchunk][:, 128 * b : 128 * (b + 1)],
                                    ident_r,
                                )
                            nc.scalar.activation(
                                stg[:, b, :], ps.bitcast(f32), AF.Copy
                            )
                        if no_out_dma:
                            nc.sync.dma_start(
                                out=g_d[k][row0 : row0 + 128, :], in_=stg[:, 0, :])
                        else:
                            nc.sync.dma_start(
                                out=g_d[k][row0 : row0 + NT, :].rearrange(
                                    "(b p) c -> p b c", b=NB
                                ),
                                in_=stg,
                            )

                      # output layer
                      pso = dd_ps.tile([128, NT], f32, tag="ddps")
                      MM(pso[:S, :], wout_r[:, 0, :], gts[2][0],
                                       start=True, stop=False)
                      MM(pso[:S, :], wout_r[:, 1, :], gts[2][1],
                                       start=False, stop=True)
                      ot = ost.tile([128, NT], f32, tag="ot")
                      nc.scalar.activation(ot[:S, :], pso[:S, :], AF.Tanh,
                                           bias=bias_sb[:S, 28:29])
                      stg = ost.tile([128, NB, S], f32, tag="ostg")
                      for b in range(NB):
                          ps = st_ps.tile([128, C], f32, tag="stps")
                          nc.tensor.transpose(
                              ps[:, :S], ot[:S, 128 * b : 128 * (b + 1)],
                              ident[:S, :S],
                          )
                          nc.scalar.activation(stg[:, b, :], ps[:, :S], AF.Copy)
                      nc.sync.dma_start(
                          out=out_d[row0 : row0 + NT, :].rearrange(
                              "(b p) c -> p b c", b=NB
                          ),
                          in_=stg,
                      )

                    if pending[0] is not None:
                        emit_stores(*pending[0])
                    pending[0] = (gts, row0)

                if pending[0] is not None:
                    emit_stores(*pending[0])

            if repeat == 1:
                body(None)
            else:
                with tc.For_i(0, repeat, 1) as iv:
                    body(iv)

    nc.finalize()
    return nc


def _prep_host(inputs):
    """Shard activations, pre-transpose weights, pack biases."""
    f = lambda x: np.ascontiguousarray(np.asarray(x, dtype=np.float32))
    wT = {
        "wt_d1": f(inputs["w_d1"].T),
        "wt_d2": f(inputs["w_d2"].T),
        "wt_out": f(inputs["w_out"].T),
    }
    for k in (1, 2, 3):
        wT[f"wt_ih{k}"] = f(inputs[f"w_ih{k}"].T)
        wT[f"wt_hh{k}"] = f(inputs[f"w_hh{k}"].T)

    bias = np.zeros((128, 35), np.float32)
    bias[:, 0] = inputs["b_d1"][0:128]
    bias[:, 1] = inputs["b_d1"][128:256]
    bias[:, 2] = inputs["b_d2"][0:128]
    bias[:, 3] = inputs["b_d2"][128:256]
    for k in range(3):
        bih = np.asarray(inputs[f"b_ih{k+1}"], np.float32)
        bhh = np.asarray(inputs[f"b_hh{k+1}"], np.float32)
        brz = bih[: 2 * C] + bhh[: 2 * C]
        bb = 4 + 8 * k
        for m in range(4):
            bias[:, bb + m] = brz[128 * m : 128 * (m + 1)]
        bias[:, bb + 4] = bih[2 * C : 2 * C + 128]
        bias[:, bb + 5] = bih[2 * C + 128 :]
        bias[:, bb + 6] = bhh[2 * C : 2 * C + 128]
        bias[:, bb + 7] = bhh[2 * C + 128 :]
        bias[:, 29 + 2 * k + 0] = -brz[C + 0 : C + 128]
        bias[:, 29 + 2 * k + 1] = -brz[C + 128 : C + 256]
    bias[:S, 28] = inputs["b_out"]
    ident = np.eye(128, dtype=np.float32)

    in_maps = []
    for c in range(NCORES):
        sl = slice(c * BLOC, (c + 1) * BLOC)
        m = {
            "cond": f(inputs["cond"][sl]),
            "prev": f(inputs["prev"][sl]),
            "h1": f(inputs["h1"][sl]),
            "h2": f(inputs["h2"][sl]),
            "h3": f(inputs["h3"][sl]),
            "biases": bias,
            "ident": ident,
        }
        m.update(wT)
        in_maps.append(m)
    return in_maps


def kernel(**inputs):
    from concourse.bass_utils import run_bass_kernel_spmd

    if "nc" not in _CACHE:
        _CACHE["nc"] = _build(repeat=1)
    nc = _CACHE["nc"]
    in_maps = _prep_host(inputs)
    res = run_bass_kernel_spmd(nc, in_maps, list(range(NCORES)))
    outs = res.results
    out = np.concatenate([outs[c]["out"] for c in range(NCORES)], axis=0)
    g1 = np.concatenate([outs[c]["g1"] for c in range(NCORES)], axis=0)
    g2 = np.concatenate([outs[c]["g2"] for c in range(NCORES)], axis=0)
    g3 = np.concatenate([outs[c]["g3"] for c in range(NCORES)], axis=0)
    return out, g1, g2, g3
